# revision 10
# baseline (speedup 1.0000x reference)
"""Blockwise reconditioner (block-16 normalization) on 8 Trainium2 cores.

Math per row r, block g (block size 16):
    mean = mean(x[r, 16g:16g+16])
    var  = sum((x - mean)^2) / 15          (unbiased, ddof=1)
    out  = (x - mean) / sqrt(var + 1e-5) * scales[g] + shifts[g]

Implemented as out = x * a + b with per-block coefficients
    a = scales[g] / sqrt(var + eps)
    b = shifts[g] - mean * a
using raw = sum(x^2) - sum(x)^2/16, var = raw/15.

Sharding: data-parallel over rows; each of 8 cores handles a [512, 8192]
shard.  Per-core pipeline (Tile framework), per row-tile [128, 8192]:
  - DMA in by column chunks (8KB/partition descriptors)
  - s1 = grouped reduce_sum(x) per 16-block     (DVE)
  - sq = x^2                                    (ACT)
  - s2 = grouped reduce_sum(sq)                 (GpSimd)
  - coefficient math on [128, 512]              (DVE + ACT sqrt)
  - apply: x*a (DVE), +b split DVE/GpSimd, DMA out per chunk
Engine budget per core (cost model): DMA ~96us (bound), DVE ~90us,
GpSimd ~90us, ACT ~35us.
"""

import sys

import numpy as np

for _p in ("/opt/trn_rl_repo",):
    if _p not in sys.path:
        sys.path.insert(0, _p)

import concourse.bacc as bacc
import concourse.bass as bass
import concourse.tile as tile
from concourse import mybir
from concourse.bass_utils import run_bass_kernel_spmd

F32 = mybir.dt.float32
ALU = mybir.AluOpType

N_CORES = 8
B_FULL = 4096          # total rows
N = 8192               # features
BLOCK = 16
NB = N // BLOCK        # 512 blocks
EPS = 1e-5
R = B_FULL // N_CORES  # 512 rows per core

CW = 2048              # column chunk width
# blocks (of CW // BLOCK per chunk) whose apply (mul+add) runs on GpSimd;
# the rest runs on DVE.  Balances DVE (reduces + coeffs + apply share)
# against GpSimd (apply share at 0.42 efficiency).
APPLY_GP_BLOCKS = 100


def build_nc(rows: int = R, cols: int = N, cw: int = CW,
             apply_gp_blocks: int = APPLY_GP_BLOCKS) -> bass.Bass:
    nb = cols // BLOCK
    nrt = rows // 128
    ncc = cols // cw
    nbw = cw // BLOCK

    # Bacc (not raw Bass): its compile() pass splits multi-semaphore waits
    # into InstEventSemaphore chains — TRN2 allows at most 1 wait per
    # instruction and walrus codegen rejects more ("Too many sync wait").
    nc = bacc.Bacc("TRN2", target_bir_lowering=False, debug=False,
                   num_devices=N_CORES)
    x = nc.declare_dram_parameter("x", [rows, cols], F32, isOutput=False)
    scales = nc.declare_dram_parameter("scales", [nb], F32, isOutput=False)
    shifts = nc.declare_dram_parameter("shifts", [nb], F32, isOutput=False)
    out = nc.declare_dram_parameter("out", [rows, cols], F32, isOutput=True)

    with tile.TileContext(nc) as tc:
        with (
            tc.tile_pool(name="singles", bufs=1) as singles,
            tc.tile_pool(name="xp", bufs=3) as xp,
            tc.tile_pool(name="sqp", bufs=3) as sqp,
            tc.tile_pool(name="wsp", bufs=2) as wsp,
        ):
            sc = singles.tile([128, nb], F32)
            sh = singles.tile([128, nb], F32)
            nc.gpsimd.dma_start(out=sc[:, :], in_=scales[:].partition_broadcast(128))
            nc.gpsimd.dma_start(out=sh[:, :], in_=shifts[:].partition_broadcast(128))
            eps_t = singles.tile([128, 1], F32)
            nc.vector.memset(eps_t[:, :], EPS)

            for rt in range(nrt):
                r0 = rt * 128
                xt = xp.tile([128, cols], F32, tag="x")
                x3 = xt[:, :].rearrange("p (g b) -> p g b", b=BLOCK)
                # packed per-row-tile workspace: 10 slots of [128, nb]
                ws = wsp.tile([128, 10 * nb], F32, tag="ws")
                s1 = ws[:, 0 * nb : 1 * nb]
                s2 = ws[:, 1 * nb : 2 * nb]
                mm = ws[:, 2 * nb : 3 * nb]
                raw = ws[:, 3 * nb : 4 * nb]
                sd = ws[:, 4 * nb : 5 * nb]
                rstd = ws[:, 5 * nb : 6 * nb]
                rscr = ws[:, 6 * nb : 7 * nb]
                a = ws[:, 7 * nb : 8 * nb]
                u = ws[:, 8 * nb : 9 * nb]
                b = ws[:, 9 * nb : 10 * nb]

                for c in range(ncc):
                    sl = slice(c * cw, (c + 1) * cw)
                    bsl = slice(c * nbw, (c + 1) * nbw)
                    nc.sync.dma_start(out=xt[:, sl], in_=x[r0 : r0 + 128, sl])
                    nc.vector.tensor_reduce(
                        out=s1[:, bsl], in_=x3[:, bsl, :],
                        op=ALU.add, axis=mybir.AxisListType.X,
                    )
                    sq = sqp.tile([128, cw], F32, tag="sq")
                    nc.scalar.square(out=sq[:, :], in_=xt[:, sl])
                    sq3 = sq[:, :].rearrange("p (g b) -> p g b", b=BLOCK)
                    nc.vector.tensor_reduce(
                        out=s2[:, bsl], in_=sq3,
                        op=ALU.add, axis=mybir.AxisListType.X,
                    )

                # coefficients, full row-tile width
                nc.scalar.square(out=mm, in_=s1)
                nc.vector.scalar_tensor_tensor(
                    out=raw, in0=mm, scalar=-1.0 / BLOCK, in1=s2,
                    op0=ALU.mult, op1=ALU.add,
                )
                nc.scalar.activation(
                    out=sd, in_=raw, func=mybir.ActivationFunctionType.Sqrt,
                    bias=eps_t[:, :], scale=1.0 / (BLOCK - 1),
                )
                nc.vector.reciprocal_approx_accurate(out=rstd, in_=sd, scratch=rscr)
                nc.vector.tensor_mul(out=a, in0=sc[:, :], in1=rstd)
                nc.vector.tensor_mul(out=u, in0=s1, in1=a)
                nc.vector.scalar_tensor_tensor(
                    out=b, in0=u, scalar=-1.0 / BLOCK, in1=sh[:, :],
                    op0=ALU.mult, op1=ALU.add,
                )

                for c in range(ncc):
                    sl = slice(c * cw, (c + 1) * cw)
                    blo, bhi = c * nbw, (c + 1) * nbw
                    x3c = x3[:, blo:bhi, :]
                    a3 = a[:, blo:bhi].unsqueeze(2).broadcast_to((128, nbw, BLOCK))
                    b3 = b[:, blo:bhi].unsqueeze(2).broadcast_to((128, nbw, BLOCK))
                    # two independent lanes: GpSimd handles blocks [0:g],
                    # DVE handles [g:] — no cross-engine deps inside a chunk
                    g = max(0, min(apply_gp_blocks, nbw))
                    if g > 0:
                        nc.gpsimd.tensor_mul(
                            out=x3c[:, :g, :], in0=x3c[:, :g, :], in1=a3[:, :g, :]
                        )
                        nc.gpsimd.tensor_add(
                            out=x3c[:, :g, :], in0=x3c[:, :g, :], in1=b3[:, :g, :]
                        )
                    if g < nbw:
                        nc.vector.tensor_mul(
                            out=x3c[:, g:, :], in0=x3c[:, g:, :], in1=a3[:, g:, :]
                        )
                        nc.vector.tensor_add(
                            out=x3c[:, g:, :], in0=x3c[:, g:, :], in1=b3[:, g:, :]
                        )
                    nc.sync.dma_start(out=out[r0 : r0 + 128, sl], in_=xt[:, sl])
    nc.compile()
    return nc


_NC_CACHE: dict = {}


def _get_nc() -> bass.Bass:
    if "nc" not in _NC_CACHE:
        _NC_CACHE["nc"] = build_nc()
    return _NC_CACHE["nc"]


def run_sharded(x, scales, shifts, trace: bool = False):
    """Run the SPMD kernel on 8 cores. Returns (out, BassKernelResults)."""
    x = np.ascontiguousarray(np.asarray(x, dtype=np.float32))
    scales = np.ascontiguousarray(np.asarray(scales, dtype=np.float32))
    shifts = np.ascontiguousarray(np.asarray(shifts, dtype=np.float32))
    assert x.shape == (B_FULL, N), x.shape
    nc = _get_nc()
    in_maps = [
        {"x": x[i * R : (i + 1) * R], "scales": scales, "shifts": shifts}
        for i in range(N_CORES)
    ]
    res = run_bass_kernel_spmd(nc, in_maps, core_ids=list(range(N_CORES)), trace=trace)
    outs = [np.asarray(m["out"]) for m in res.results]
    return np.concatenate(outs, axis=0), res


def kernel(x, scales, shifts):
    out, _ = run_sharded(x, scales, shifts, trace=False)
    return out


# revision 24
# speedup vs baseline: 1.3111x; 1.3111x over previous
"""Blockwise reconditioner (block-16 normalization) on 8 Trainium2 cores.

Math per row r, block g (block size 16):
    mean = mean(x[r, 16g:16g+16])
    var  = sum((x - mean)^2) / 15          (unbiased, ddof=1)
    out  = (x - mean) / sqrt(var + 1e-5) * scales[g] + shifts[g]

Implemented as out = x * a + b with per-block coefficients
    a = scales[g] / sqrt(var + eps)
    b = shifts[g] - mean * a
using raw = sum(x^2) - sum(x)^2/16, var = raw/15.

Sharding: data-parallel over rows; each of 8 cores handles a [512, 8192]
shard.  Per-core pipeline (Tile framework), per row-tile [128, 8192],
processed per 2048-column chunk:
  - DMA in (8KB/partition descriptors)
  - s1 = grouped reduce_sum(x) per 16-block      (DVE)
  - s2 = sum(x^2) per block on the TensorEngine: PE-transpose 128x128
    sub-blocks to PSUM, ACT squares PSUM->SBUF, masked fp32 matmuls
    (contraction over partitions = features) accumulate the 8 block-sums
    of each sub-block into a [128 blocks, 128 rows] PSUM tile, PE-flip
    back to row-major.  This keeps the second reduction off the DVE,
    which is the critical engine.
  - coefficient math on [128, 128] slices        (DVE + ACT sqrt)
  - apply out = x*a + b: two DVE passes (in-place), DMA out
Notes from HW measurement: DVE is the bottleneck (~126us busy/core);
GpSimd streaming contends with DVE for SBUF bandwidth (concurrent DVE
ops stretch to the GpSimd op duration), so GpSimd is left idle.  DMA
queues run in parallel at ~83us aggregate; ACT ~58us; PE ~90us.
Measured HW exec ~160us/core (allDVE fallback ~180us).
"""

import sys

import numpy as np

for _p in ("/opt/trn_rl_repo",):
    if _p not in sys.path:
        sys.path.insert(0, _p)

import concourse.bacc as bacc
import concourse.bass as bass
import concourse.tile as tile
from concourse import mybir
from concourse.bass_utils import run_bass_kernel_spmd

F32 = mybir.dt.float32
ALU = mybir.AluOpType

N_CORES = 8
B_FULL = 4096          # total rows
N = 8192               # features
BLOCK = 16
NB = N // BLOCK        # 512 blocks
EPS = 1e-5
R = B_FULL // N_CORES  # 512 rows per core

CW = 2048              # column chunk width
# Of every APPLY_GP_DEN consecutive chunks, the first APPLY_GP_NUM get their
# apply (mul+add) on GpSimd; the rest on DVE.  (0, 1) = all-DVE.
APPLY_GP_NUM = 0
APPLY_GP_DEN = 1


def build_nc(rows: int = R, cols: int = N, cw: int = CW,
             apply_gp_num: int = APPLY_GP_NUM,
             apply_gp_den: int = APPLY_GP_DEN,
             pe_stats: bool = True) -> bass.Bass:
    nb = cols // BLOCK
    nrt = rows // 128
    ncc = cols // cw
    nbw = cw // BLOCK

    # Bacc (not raw Bass): its compile() pass splits multi-semaphore waits
    # into InstEventSemaphore chains — TRN2 allows at most 1 wait per
    # instruction and walrus codegen rejects more ("Too many sync wait").
    nc = bacc.Bacc("TRN2", target_bir_lowering=False, debug=False,
                   num_devices=N_CORES)
    x = nc.declare_dram_parameter("x", [rows, cols], F32, isOutput=False)
    scales = nc.declare_dram_parameter("scales", [nb], F32, isOutput=False)
    shifts = nc.declare_dram_parameter("shifts", [nb], F32, isOutput=False)
    if pe_stats:
        ident = nc.declare_dram_parameter("ident", [128, 128], F32, isOutput=False)
        # maskall[f, k*128 + g] = 1 iff g == 8k + f//16: matmul k of a chunk
        # accumulates sub-block k's 8 block-sums into output partitions
        # 8k..8k+8 (PE out base partition must be 0 — masks route instead).
        mask = nc.declare_dram_parameter(
            "maskall", [128, (cw // 128) * 128], F32, isOutput=False)
    out = nc.declare_dram_parameter("out", [rows, cols], F32, isOutput=True)

    with tile.TileContext(nc) as tc:
        with (
            tc.tile_pool(name="singles", bufs=1) as singles,
            tc.tile_pool(name="xp", bufs=3) as xp,
            tc.tile_pool(name="sqp", bufs=3) as sqp,
            tc.tile_pool(name="wsp", bufs=2) as wsp,
            tc.tile_pool(name="psA", bufs=2, space="PSUM") as psA,
            tc.tile_pool(name="psB", bufs=2, space="PSUM") as psB,
            tc.tile_pool(name="psF", bufs=2, space="PSUM") as psF,
            tc.tile_pool(name="stp2", bufs=2) as stp2,
        ):
            sc = singles.tile([128, nb], F32)
            sh = singles.tile([128, nb], F32)
            nc.gpsimd.dma_start(out=sc[:, :], in_=scales[:].partition_broadcast(128))
            nc.gpsimd.dma_start(out=sh[:, :], in_=shifts[:].partition_broadcast(128))
            eps_t = singles.tile([128, 1], F32)
            nc.vector.memset(eps_t[:, :], EPS)
            if pe_stats:
                ident_sb = singles.tile([128, 128], F32)
                mask_sb = singles.tile([128, (cw // 128) * 128], F32)
                nc.sync.dma_start(out=ident_sb[:, :], in_=ident[:, :])
                nc.sync.dma_start(out=mask_sb[:, :], in_=mask[:, :])

            for rt in range(nrt):
                r0 = rt * 128
                xt = xp.tile([128, cols], F32, tag="x")
                x3 = xt[:, :].rearrange("p (g b) -> p g b", b=BLOCK)
                # packed per-row-tile workspace: 10 slots of [128, nb]
                ws = wsp.tile([128, 10 * nb], F32, tag="ws")
                s1 = ws[:, 0 * nb : 1 * nb]
                s2 = ws[:, 1 * nb : 2 * nb]
                mm = ws[:, 2 * nb : 3 * nb]
                raw = ws[:, 3 * nb : 4 * nb]
                sd = ws[:, 4 * nb : 5 * nb]
                rstd = ws[:, 5 * nb : 6 * nb]
                rscr = ws[:, 6 * nb : 7 * nb]
                a = ws[:, 7 * nb : 8 * nb]
                u = ws[:, 8 * nb : 9 * nb]
                b = ws[:, 9 * nb : 10 * nb]

                def coeffs(bsl):
                    # per-block a = scales/sqrt(var+eps), b = shifts - mean*a
                    nc.scalar.square(out=mm[:, bsl], in_=s1[:, bsl])
                    nc.vector.scalar_tensor_tensor(
                        out=raw[:, bsl], in0=mm[:, bsl], scalar=-1.0 / BLOCK,
                        in1=s2[:, bsl], op0=ALU.mult, op1=ALU.add,
                    )
                    nc.scalar.activation(
                        out=sd[:, bsl], in_=raw[:, bsl],
                        func=mybir.ActivationFunctionType.Sqrt,
                        bias=eps_t[:, :], scale=1.0 / (BLOCK - 1),
                    )
                    nc.vector.reciprocal_approx_accurate(
                        out=rstd[:, bsl], in_=sd[:, bsl], scratch=rscr[:, bsl])
                    nc.vector.tensor_mul(out=a[:, bsl], in0=sc[:, bsl],
                                         in1=rstd[:, bsl])
                    nc.vector.tensor_mul(out=u[:, bsl], in0=s1[:, bsl],
                                         in1=a[:, bsl])
                    nc.vector.scalar_tensor_tensor(
                        out=b[:, bsl], in0=u[:, bsl], scalar=-1.0 / BLOCK,
                        in1=sh[:, bsl], op0=ALU.mult, op1=ALU.add,
                    )

                def apply_chunk(rt, c):
                    sl = slice(c * cw, (c + 1) * cw)
                    blo, bhi = c * nbw, (c + 1) * nbw
                    x3c = x3[:, blo:bhi, :]
                    a3 = a[:, blo:bhi].unsqueeze(2).broadcast_to(
                        (128, nbw, BLOCK))
                    b3 = b[:, blo:bhi].unsqueeze(2).broadcast_to(
                        (128, nbw, BLOCK))
                    on_gp = (rt * ncc + c) % apply_gp_den < apply_gp_num
                    eng = nc.gpsimd if on_gp else nc.vector
                    eng.tensor_mul(out=x3c, in0=x3c, in1=a3)
                    eng.tensor_add(out=x3c, in0=x3c, in1=b3)
                    nc.sync.dma_start(out=out[r0 : r0 + 128, sl],
                                      in_=xt[:, sl])

                spc = cw // 128  # 128-col sub-blocks per chunk
                for c in range(ncc):
                    sl = slice(c * cw, (c + 1) * cw)
                    bsl = slice(c * nbw, (c + 1) * nbw)
                    nc.sync.dma_start(out=xt[:, sl], in_=x[r0 : r0 + 128, sl])
                    nc.vector.tensor_reduce(
                        out=s1[:, bsl], in_=x3[:, bsl, :],
                        op=ALU.add, axis=mybir.AxisListType.X,
                    )
                    if not pe_stats:
                        sq = sqp.tile([128, cw], F32, tag="sq")
                        nc.scalar.square(out=sq[:, :], in_=xt[:, sl])
                        sq3 = sq[:, :].rearrange("p (g b) -> p g b", b=BLOCK)
                        nc.vector.tensor_reduce(
                            out=s2[:, bsl], in_=sq3,
                            op=ALU.add, axis=mybir.AxisListType.X,
                        )
                        continue
                    # s2 on the TensorEngine: transpose x sub-blocks to PSUM,
                    # square them PSUM->SBUF on ACT, block-sum via masked
                    # fp32 matmuls (contraction along partitions = features)
                    # accumulating into one [128, 128] PSUM tile per chunk,
                    # then flip the [block, row] stats back to row-major.
                    sqT = sqp.tile([128, cw], F32, tag="sqT")
                    for half in range(cw // 1024):
                        xT = psA.tile([128, 1024], F32, tag="xT")
                        for j in range(8):
                            col0 = c * cw + half * 1024 + j * 128
                            nc.tensor.transpose(
                                xT[:, j * 128 : (j + 1) * 128],
                                xt[:, col0 : col0 + 128],
                                ident_sb[:, :],
                            )
                        nc.scalar.square(
                            out=sqT[:, half * 1024 : (half + 1) * 1024],
                            in_=xT[:, :],
                        )
                    s2c = psB.tile([128, 128], F32, tag="s2c")
                    for k in range(spc):
                        nc.tensor.matmul(
                            s2c[:, :],
                            mask_sb[:, k * 128 : (k + 1) * 128],
                            sqT[:, k * 128 : (k + 1) * 128],
                            start=(k == 0), stop=(k == spc - 1),
                        )
                    st = stp2.tile([128, 128], F32, tag="st")
                    nc.scalar.copy(out=st[:, :], in_=s2c[:, :])
                    fp = psF.tile([128, 128], F32, tag="fp")
                    nc.tensor.transpose(fp[:, :], st[:, :], ident_sb[:, :])
                    nc.scalar.copy(out=s2[:, bsl], in_=fp[:, :])
                    # coefficients + apply for this chunk right away
                    coeffs(bsl)
                    apply_chunk(rt, c)

                if not pe_stats:
                    coeffs(slice(0, nb))
                    for c in range(ncc):
                        apply_chunk(rt, c)
    nc.compile()
    return nc


def aux_inputs(cw: int = CW) -> dict:
    """Constant tensors fed alongside the real inputs (PE-stats variant)."""
    spc = cw // 128
    maskall = np.zeros((128, spc * 128), np.float32)
    for k in range(spc):
        for f in range(128):
            maskall[f, k * 128 + 8 * k + f // BLOCK] = 1.0
    return {"ident": np.eye(128, dtype=np.float32), "maskall": maskall}


_NC_CACHE: dict = {}


def _get_nc() -> bass.Bass:
    if "nc" not in _NC_CACHE:
        _NC_CACHE["nc"] = build_nc()
    return _NC_CACHE["nc"]


def run_sharded(x, scales, shifts, trace: bool = False):
    """Run the SPMD kernel on 8 cores. Returns (out, BassKernelResults)."""
    x = np.ascontiguousarray(np.asarray(x, dtype=np.float32))
    scales = np.ascontiguousarray(np.asarray(scales, dtype=np.float32))
    shifts = np.ascontiguousarray(np.asarray(shifts, dtype=np.float32))
    assert x.shape == (B_FULL, N), x.shape
    nc = _get_nc()
    in_maps = [
        {"x": x[i * R : (i + 1) * R], "scales": scales, "shifts": shifts,
         **aux_inputs()}
        for i in range(N_CORES)
    ]
    res = run_bass_kernel_spmd(nc, in_maps, core_ids=list(range(N_CORES)), trace=trace)
    outs = [np.asarray(m["out"]) for m in res.results]
    return np.concatenate(outs, axis=0), res


def kernel(x, scales, shifts):
    out, _ = run_sharded(x, scales, shifts, trace=False)
    return out
